# revision 39
# baseline (speedup 1.0000x reference)
"""MLA QKV projection kernel for Trainium2 (8 NeuronCores, Bass/Tile).

Computes the DeepSeek-MLA q/k/v projection:
  q  = rmsnorm(hs @ q_a_w.T) @ q_b_w.T          -> [b, H, s, 192]  (pe cols rope-interleaved)
  ckv = hs @ kv_a_w.T ; compressed, k_pe = split
  kv = rmsnorm(compressed) @ kv_b_w.T           -> k_nope, v
  out = concat([q, concat(k_nope, k_pe), pad(v)], head axis)  -> [b, 3H, s, 192]

Strategy: pure data-parallel over tokens (8192 tokens -> 1024/core); all
weights replicated. All matmul operands are bf16 (same 1 cycle/row PE rate
as fp32r on TRN2, half the HBM traffic and SBUF footprint); accumulation is
f32 in PSUM. Each weight chunk is streamed from HBM exactly once and used
for all 8 token m-tiles (two PSUM-tile halves of 4 m each, double-buffered),
keeping the tensor engine continuously fed so it ramps to the full 2.4 GHz
p-state. The RMSNorm layernorm weights are folded into the B projection
weights on the host; the per-token rsqrt scale is applied to the B-GEMM
outputs at PSUM-eviction time (scale commutes with the GEMM). The rope
interleave permutation is folded into the rows of q_b_w / kv_a_w on the
host.
"""

import sys
import types

import numpy as np

# ---- constants (hardcoded problem shape) ----
H = 32
D_NOPE = 128
D_ROPE = 64
D_Q = 192
D_V = 128
R_KV = 512
RQ = 1536
DMODEL = 4096
EPS = 1e-6
B, S = 2, 4096
NTOK = B * S            # 8192
NCORES = 8
TPC = NTOK // NCORES    # 1024 tokens per core
MT = TPC // 128         # 8 m-tiles
KT = DMODEL // 128      # 32 k-tiles for the A GEMM

A_COLS = RQ + R_KV + D_ROPE   # 2112 = 4*512 (normed) + 64 (k_pe)
A_NORM = RQ + R_KV            # 2048
A_CHUNKS = ((0, 512), (512, 512), (1024, 512), (1536, 512), (2048, 64))
QOUT = H * D_Q                # 6144
KVOUT = H * (D_NOPE + D_V)    # 8192
QCH = QOUT // 512             # 12 B-q chunks
KVCH = KVOUT // 512           # 16 B-kv chunks
QKT = RQ // 128               # 12
KVKT = R_KV // 128            # 4
FT = A_NORM // 128            # 16 f-tiles to transpose


def _ensure_env():
    for p in ("/opt/trn_rl_repo", "/root/.axon_site"):
        if p not in sys.path:
            sys.path.insert(0, p)
    # bass_utils under axon wants antenv.axon_hooks; provide a stub so
    # trace=False runs never trip on the missing module.
    if "antenv.axon_hooks" not in sys.modules:
        try:
            import antenv  # noqa: F401
            import antenv.axon_hooks  # noqa: F401
        except ImportError:
            mod = types.ModuleType("antenv.axon_hooks")
            mod._hook = None
            mod.set_axon_ntff_profile_hook = lambda h: setattr(mod, "_hook", h)
            mod.get_axon_ntff_profile_hook = lambda: mod._hook
            sys.modules["antenv.axon_hooks"] = mod
            try:
                import antenv
                antenv.axon_hooks = mod
            except ImportError:
                pass


def _perm64():
    # inverse view of x.reshape(32,2).swapaxes ->  y[k] = x[2*(k%32) + k//32]
    return np.array([2 * (k % 32) + k // 32 for k in range(64)], dtype=np.int64)


_CACHE = {}


def _build():
    if "nc" in _CACHE:
        return _CACHE["nc"]
    _ensure_env()
    from concourse import bacc
    import concourse.mybir as mybir
    import concourse.tile as tile
    from concourse.masks import make_identity

    F32 = mybir.dt.float32
    BF16 = mybir.dt.bfloat16
    AF = mybir.ActivationFunctionType
    ALU = mybir.AluOpType
    AX = mybir.AxisListType

    nc = bacc.Bacc("TRN2", target_bir_lowering=False, debug=False)
    hsT_d = nc.dram_tensor("hsT", [DMODEL, TPC], BF16, kind="ExternalInput")
    waT_d = nc.dram_tensor("waT", [DMODEL, A_COLS], BF16, kind="ExternalInput")
    qbT_d = nc.dram_tensor("qbT", [RQ, QOUT], BF16, kind="ExternalInput")
    kvbT_d = nc.dram_tensor("kvbT", [R_KV, KVOUT], BF16, kind="ExternalInput")
    out_d = nc.dram_tensor("out", [3 * H, TPC, D_Q], F32, kind="ExternalOutput")

    with tile.TileContext(nc) as tc:
        with tc.tile_pool(name="persist", bufs=1) as persist:
            a_sb = persist.tile([128, MT, A_NORM], BF16)        # 32KB/p token-major A out
            kpe_sb = persist.tile([128, MT, D_ROPE], F32)       # 2KB/p
            stats = persist.tile([128, MT, 4], F32)             # per-chunk sumsq
            s_q = persist.tile([128, MT], F32)
            s_kv = persist.tile([128, MT], F32)

            # ---------------- phase A: fused A GEMM ----------------
            # Each 512-col weight chunk is DMA'd once and used by all 8
            # m-tiles: two psum halves (m 0-3, m 4-7) so the pool
            # double-buffers across chunks with 4 banks each.
            with tc.tile_pool(name="hst", bufs=1) as hstp, \
                 tc.tile_pool(name="wa", bufs=2 * KT + 4) as wap, \
                 tc.tile_pool(name="sqs", bufs=2) as sqsp, \
                 tc.tile_pool(name="psA", bufs=2, space="PSUM") as psp:
                hst = [hstp.tile([128, TPC], BF16, name=f"hst{k}", tag=f"hst{k}")
                       for k in range(KT)]
                # two half-column DMAs per tile, half0 (m 0-3) streamed first so
                # the first chunk's g0 matmuls start as early as possible
                for half in range(2):
                    for k in range(KT):
                        eng = nc.scalar if k % 2 == 0 else nc.gpsimd
                        eng.dma_start(out=hst[k][:, half * 512:(half + 1) * 512],
                                      in_=hsT_d[k * 128:(k + 1) * 128, half * 512:(half + 1) * 512])
                for c, (col0, w) in enumerate(A_CHUNKS):
                    ps0 = psp.tile([128, 4, 512], F32, tag="ps")
                    ps1 = psp.tile([128, 4, 512], F32, tag="ps")
                    wts = []
                    for k in range(KT):
                        wa_t = wap.tile([128, 512], BF16, tag="wa_t")
                        nc.sync.dma_start(out=wa_t[:, :w], in_=waT_d[k * 128:(k + 1) * 128, col0:col0 + w])
                        wts.append(wa_t)
                        for mi in range(4):
                            nc.tensor.matmul(ps0[:, mi, :w], hst[k][:, mi * 128:(mi + 1) * 128],
                                             wa_t[:, :w], start=(k == 0), stop=(k == KT - 1))
                    for k in range(KT):
                        for mi in range(4):
                            m = 4 + mi
                            nc.tensor.matmul(ps1[:, mi, :w], hst[k][:, m * 128:(m + 1) * 128],
                                             wts[k][:, :w], start=(k == 0), stop=(k == KT - 1))
                    # evictions split across the (otherwise idle) vector engine
                    # and scalar, per mi, so no single engine queue backlogs
                    for g, ps in ((0, ps0), (1, ps1)):
                        for mi in range(4):
                            m = g * 4 + mi
                            if c < 4:
                                asl = a_sb[:, m, col0:col0 + w]
                                scr = sqsp.tile([128, 512], F32, tag="scr")
                                if mi % 2 == 0:
                                    nc.vector.tensor_copy(asl, ps[:, mi, :w])
                                else:
                                    nc.scalar.activation(asl, ps[:, mi, :w], AF.Copy)
                                nc.scalar.activation(scr[:, :w], ps[:, mi, :w], AF.Square,
                                                     accum_out=stats[:, m, c:c + 1])
                            else:
                                if mi % 2 == 0:
                                    nc.vector.tensor_copy(kpe_sb[:, m, :], ps[:, mi, :w])
                                else:
                                    nc.scalar.activation(kpe_sb[:, m, :], ps[:, mi, :w], AF.Copy)

            # per-token rsqrt scales (all m at once)
            with tc.tile_pool(name="scl", bufs=1) as sclp:
                for (dst, c0, c1, dim) in ((s_q, 0, 3, RQ), (s_kv, 3, 4, R_KV)):
                    t = sclp.tile([128, MT], F32)
                    nc.vector.reduce_sum(out=t, in_=stats[:, :, c0:c1], axis=AX.X)
                    nc.vector.tensor_scalar(out=t, in0=t, scalar1=1.0 / dim, scalar2=EPS,
                                            op0=ALU.mult, op1=ALU.add)
                    nc.vector.reciprocal(t, t)
                    nc.scalar.activation(dst[:, :], t, AF.Sqrt)

            # ---------------- phase T: transposes,  phase B: B GEMMs ----------------
            with tc.tile_pool(name="at", bufs=1) as atp:
                at_q = atp.tile([128, QKT, TPC], BF16)          # 24KB/p
                at_kv = atp.tile([128, KVKT, TPC], BF16)        # 8KB/p
                ident = atp.tile([128, 128], BF16)
                make_identity(nc, ident)

                def tok_view(h, g, d0, w):
                    # DRAM view [p, mi, w] matching ev tiles' [partition, mi, cols]
                    return out_d[h, g * 512:(g + 1) * 512, d0:d0 + w].rearrange(
                        "(mi p) w -> p mi w", p=128)

                # transposes: a_sb [tok, f] -> at [f, tok]; psum->sbuf copies
                # split across vector and scalar engines, 4 f-tiles per copy
                with tc.tile_pool(name="psT", bufs=2, space="PSUM") as psT:
                    # kv f-tiles (fb 3) first: the kv B GEMM runs before q
                    for fb in (3, 0, 1, 2):
                        pt = psT.tile([128, MT, 512], BF16, tag="pt")
                        for fi in range(4):
                            f = fb * 4 + fi
                            for m in range(MT):
                                nc.tensor.transpose(pt[:, m, fi * 128:(fi + 1) * 128],
                                                    a_sb[:, m, f * 128:(f + 1) * 128], ident)
                        for m in range(MT):
                            src = pt[:, m, :].rearrange("p (a b) -> p a b", b=128)
                            if fb < 3:
                                dst = at_q[:, fb * 4:(fb + 1) * 4, m * 128:(m + 1) * 128]
                            else:
                                dst = at_kv[:, 0:4, m * 128:(m + 1) * 128]
                            if m % 2 == 0:
                                nc.vector.tensor_copy(dst, src)
                            else:
                                nc.scalar.activation(dst, src, AF.Copy)

                out_rings = (nc.gpsimd, nc.scalar)
                ring_i = [0]

                def out_dma(dst, src):
                    out_rings[ring_i[0] % 2].dma_start(out=dst, in_=src)
                    ring_i[0] += 1

                with tc.tile_pool(name="wb", bufs=2 * QKT + 2) as wbp, \
                     tc.tile_pool(name="ev", bufs=4) as evp, \
                     tc.tile_pool(name="psB", bufs=2, space="PSUM") as psp:
                    # kv B GEMM first: its small-packet output drain then
                    # overlaps the q B GEMM instead of forming the kernel tail
                    for c in range(KVCH):                   # kv chunks (512 cols = 2 heads)
                        ps0 = psp.tile([128, 4, 512], F32, tag="ps")
                        ps1 = psp.tile([128, 4, 512], F32, tag="ps")
                        wts = []
                        for k in range(KVKT):
                            wb_t = wbp.tile([128, 512], BF16, tag="wb_t")
                            nc.sync.dma_start(out=wb_t, in_=kvbT_d[k * 128:(k + 1) * 128, c * 512:(c + 1) * 512])
                            wts.append(wb_t)
                            for mi in range(4):
                                nc.tensor.matmul(ps0[:, mi, :], at_kv[:, k, mi * 128:(mi + 1) * 128],
                                                 wb_t, start=(k == 0), stop=(k == KVKT - 1))
                        for k in range(KVKT):
                            for mi in range(4):
                                m = 4 + mi
                                nc.tensor.matmul(ps1[:, mi, :], at_kv[:, k, m * 128:(m + 1) * 128],
                                                 wts[k], start=(k == 0), stop=(k == KVKT - 1))
                        # fused eviction: k heads as contiguous [nope|pe]
                        # 192-col rows and v heads as full 192-col rows (pad
                        # cols unwritten; the host zeroes the v pad after the
                        # gather) so every output DMA row is contiguous in
                        # DRAM and packets aggregate to ~96KB
                        evs = []
                        for g, ps in ((0, ps0), (1, ps1)):
                            ev2 = evp.tile([128, 4, 384], F32, tag="ev2")
                            evv = evp.tile([128, 4, 384], F32, tag="evv")
                            evs.append((ev2, evv))
                            for mi in range(4):
                                m = g * 4 + mi
                                psh = ps[:, mi, :].rearrange("p (h x) -> p h x", x=256)
                                e2h = ev2[:, mi, :].rearrange("p (h y) -> p h y", y=192)
                                evh = evv[:, mi, :].rearrange("p (h y) -> p h y", y=192)
                                nc.scalar.activation(e2h[:, :, 0:128], psh[:, :, 0:128],
                                                     AF.Copy, scale=s_kv[:, m:m + 1])
                                nc.vector.tensor_scalar_mul(evh[:, :, 0:128],
                                                            psh[:, :, 128:256],
                                                            s_kv[:, m:m + 1])
                                nc.vector.tensor_copy(e2h[:, 0, 128:192], kpe_sb[:, m, :])
                                nc.gpsimd.tensor_copy(e2h[:, 1, 128:192], kpe_sb[:, m, :])
                        for g, (ev2, evv) in enumerate(evs):
                            for hh in range(2):
                                h = 2 * c + hh
                                out_dma(tok_view(H + h, g, 0, D_Q),
                                        ev2[:, :, hh * 192:(hh + 1) * 192])
                                nc.sync.dma_start(out=tok_view(2 * H + h, g, 0, D_Q),
                                                  in_=evv[:, :, hh * 192:(hh + 1) * 192])
                    for c in range(QCH):                    # q chunks (512 cols)
                        ps0 = psp.tile([128, 4, 512], F32, tag="ps")
                        ps1 = psp.tile([128, 4, 512], F32, tag="ps")
                        wts = []
                        for k in range(QKT):
                            wb_t = wbp.tile([128, 512], BF16, tag="wb_t")
                            nc.sync.dma_start(out=wb_t, in_=qbT_d[k * 128:(k + 1) * 128, c * 512:(c + 1) * 512])
                            wts.append(wb_t)
                            for mi in range(4):
                                nc.tensor.matmul(ps0[:, mi, :], at_q[:, k, mi * 128:(mi + 1) * 128],
                                                 wb_t, start=(k == 0), stop=(k == QKT - 1))
                        for k in range(QKT):
                            for mi in range(4):
                                m = 4 + mi
                                nc.tensor.matmul(ps1[:, mi, :], at_q[:, k, m * 128:(m + 1) * 128],
                                                 wts[k], start=(k == 0), stop=(k == QKT - 1))
                        evs = []
                        for g, ps in ((0, ps0), (1, ps1)):
                            ev = evp.tile([128, 4, 512], F32, tag="ev")
                            evs.append(ev)
                            for mi in range(4):
                                m = g * 4 + mi
                                if mi % 2 == 0:
                                    nc.vector.tensor_scalar_mul(ev[:, mi, :], ps[:, mi, :],
                                                                s_q[:, m:m + 1])
                                else:
                                    nc.scalar.activation(ev[:, mi, :], ps[:, mi, :], AF.Copy,
                                                         scale=s_q[:, m:m + 1])
                        for g, ev in enumerate(evs):
                            col = c * 512
                            end = col + 512
                            while col < end:                # one DMA per head-piece
                                h = col // D_Q
                                seg_end = min(end, (h + 1) * D_Q)
                                out_dma(tok_view(h, g, col - h * D_Q, seg_end - col),
                                        ev[:, :, col - c * 512:seg_end - c * 512])
                                col = seg_end

    nc.compile()
    _CACHE["nc"] = nc
    return nc


def _prep_inputs(hidden_states, q_a_w, kv_a_w, q_b_w, kv_b_w, q_a_ln_w, kv_a_ln_w):
    import ml_dtypes
    f32 = np.float32
    bf16 = ml_dtypes.bfloat16
    hs = np.asarray(hidden_states, dtype=f32).reshape(NTOK, DMODEL)
    hsT = np.ascontiguousarray(hs.T).astype(bf16)          # [4096, 8192]
    perm = _perm64()

    q_a_w = np.asarray(q_a_w, dtype=f32)
    kv_a_w = np.asarray(kv_a_w, dtype=f32)
    kv_a_pe = kv_a_w[R_KV:][perm]                          # de-interleave k_pe rows
    wa = np.concatenate([q_a_w, kv_a_w[:R_KV], kv_a_pe], axis=0)   # [2112, 4096]
    waT = np.ascontiguousarray(wa.T).astype(bf16)          # [4096, 2112]

    qb = np.asarray(q_b_w, dtype=f32) * np.asarray(q_a_ln_w, dtype=f32)[None, :]
    qb = qb.reshape(H, D_Q, RQ).copy()
    qb[:, D_NOPE:, :] = qb[:, D_NOPE + perm, :]            # de-interleave q_pe rows
    qbT = np.ascontiguousarray(qb.reshape(QOUT, RQ).T).astype(bf16)  # [1536, 6144]

    kvb = np.asarray(kv_b_w, dtype=f32) * np.asarray(kv_a_ln_w, dtype=f32)[None, :]
    kvbT = np.ascontiguousarray(kvb.T).astype(bf16)        # [512, 8192]

    in_maps = []
    for c in range(NCORES):
        in_maps.append({
            "hsT": np.ascontiguousarray(hsT[:, c * TPC:(c + 1) * TPC]),
            "waT": waT,
            "qbT": qbT,
            "kvbT": kvbT,
        })
    return in_maps


def kernel(hidden_states, q_a_w, q_b_w, kv_a_w, kv_b_w, q_a_ln_w, kv_a_ln_w,
           _trace=False):
    _ensure_env()
    from concourse.bass_utils import run_bass_kernel_spmd

    nc = _build()
    in_maps = _prep_inputs(hidden_states, q_a_w, kv_a_w, q_b_w, kv_b_w,
                           q_a_ln_w, kv_a_ln_w)
    res = run_bass_kernel_spmd(nc, in_maps, list(range(NCORES)), trace=_trace)

    out = np.empty((B, 3 * H, S, D_Q), dtype=np.float32)
    for c in range(NCORES):
        out[c // (S // TPC), :, (c % (S // TPC)) * TPC:((c % (S // TPC)) + 1) * TPC, :] = \
            res.results[c]["out"]
    out[:, 2 * H:, :, D_V:] = 0.0      # v padding is exact zeros
    if _trace:
        kernel.last_exec_time_ns = res.exec_time_ns
        kernel.last_results = res
    return out


# revision 44
# speedup vs baseline: 1.0208x; 1.0208x over previous
"""MLA QKV projection kernel for Trainium2 (8 NeuronCores, Bass/Tile).

Computes the DeepSeek-MLA q/k/v projection:
  q  = rmsnorm(hs @ q_a_w.T) @ q_b_w.T          -> [b, H, s, 192]  (pe cols rope-interleaved)
  ckv = hs @ kv_a_w.T ; compressed, k_pe = split
  kv = rmsnorm(compressed) @ kv_b_w.T           -> k_nope, v
  out = concat([q, concat(k_nope, k_pe), pad(v)], head axis)  -> [b, 3H, s, 192]

Strategy: pure data-parallel over tokens (8192 tokens -> 1024/core); all
weights replicated. All matmul operands are bf16 (same 1 cycle/row PE rate
as fp32r on TRN2, half the HBM traffic and SBUF footprint); accumulation is
f32 in PSUM. Each weight chunk is streamed from HBM exactly once and used
for all 8 token m-tiles (two PSUM-tile halves of 4 m each, double-buffered),
keeping the tensor engine continuously fed so it ramps to the full 2.4 GHz
p-state. The RMSNorm layernorm weights are folded into the B projection
weights on the host; the per-token rsqrt scale is applied to the B-GEMM
outputs at PSUM-eviction time (scale commutes with the GEMM). The rope
interleave permutation is folded into the rows of q_b_w / kv_a_w on the
host.
"""

import sys
import types

import numpy as np

# ---- constants (hardcoded problem shape) ----
H = 32
D_NOPE = 128
D_ROPE = 64
D_Q = 192
D_V = 128
R_KV = 512
RQ = 1536
DMODEL = 4096
EPS = 1e-6
B, S = 2, 4096
NTOK = B * S            # 8192
NCORES = 8
TPC = NTOK // NCORES    # 1024 tokens per core
MT = TPC // 128         # 8 m-tiles
KT = DMODEL // 128      # 32 k-tiles for the A GEMM

A_COLS = RQ + R_KV + D_ROPE   # 2112 = 4*512 (normed) + 64 (k_pe)
A_NORM = RQ + R_KV            # 2048
A_CHUNKS = ((0, 512), (512, 512), (1024, 512), (1536, 512), (2048, 64))
QOUT = H * D_Q                # 6144
KVOUT = H * (D_NOPE + D_V)    # 8192
QCH = QOUT // 512             # 12 B-q chunks
KVCH = KVOUT // 512           # 16 B-kv chunks
QKT = RQ // 128               # 12
KVKT = R_KV // 128            # 4
FT = A_NORM // 128            # 16 f-tiles to transpose


def _ensure_env():
    for p in ("/opt/trn_rl_repo", "/root/.axon_site"):
        if p not in sys.path:
            sys.path.insert(0, p)
    # bass_utils under axon wants antenv.axon_hooks; provide a stub so
    # trace=False runs never trip on the missing module.
    if "antenv.axon_hooks" not in sys.modules:
        try:
            import antenv  # noqa: F401
            import antenv.axon_hooks  # noqa: F401
        except ImportError:
            mod = types.ModuleType("antenv.axon_hooks")
            mod._hook = None
            mod.set_axon_ntff_profile_hook = lambda h: setattr(mod, "_hook", h)
            mod.get_axon_ntff_profile_hook = lambda: mod._hook
            sys.modules["antenv.axon_hooks"] = mod
            try:
                import antenv
                antenv.axon_hooks = mod
            except ImportError:
                pass


def _perm64():
    # inverse view of x.reshape(32,2).swapaxes ->  y[k] = x[2*(k%32) + k//32]
    return np.array([2 * (k % 32) + k // 32 for k in range(64)], dtype=np.int64)


_CACHE = {}


def _build():
    if "nc" in _CACHE:
        return _CACHE["nc"]
    _ensure_env()
    from concourse import bacc
    import concourse.mybir as mybir
    import concourse.tile as tile
    from concourse.masks import make_identity

    F32 = mybir.dt.float32
    BF16 = mybir.dt.bfloat16
    AF = mybir.ActivationFunctionType
    ALU = mybir.AluOpType
    AX = mybir.AxisListType

    nc = bacc.Bacc("TRN2", target_bir_lowering=False, debug=False)
    hsT_d = nc.dram_tensor("hsT", [DMODEL, TPC], BF16, kind="ExternalInput")
    waT_d = nc.dram_tensor("waT", [DMODEL, A_COLS], BF16, kind="ExternalInput")
    qbT_d = nc.dram_tensor("qbT", [RQ, QOUT], BF16, kind="ExternalInput")
    kvbT_d = nc.dram_tensor("kvbT", [R_KV, KVOUT], BF16, kind="ExternalInput")
    out_d = nc.dram_tensor("out", [3 * H, TPC, D_Q], F32, kind="ExternalOutput")

    with tile.TileContext(nc) as tc:
        with tc.tile_pool(name="persist", bufs=1) as persist:
            a_sb = persist.tile([128, MT, A_NORM], BF16)        # 32KB/p token-major A out
            kpe_sb = persist.tile([128, MT, D_ROPE], F32)       # 2KB/p
            stats = persist.tile([128, MT, 4], F32)             # per-chunk sumsq
            s_q = persist.tile([128, MT], F32)
            s_kv = persist.tile([128, MT], F32)

            # ---------------- phase A: fused A GEMM ----------------
            # Each 512-col weight chunk is DMA'd once and used by all 8
            # m-tiles: two psum halves (m 0-3, m 4-7) so the pool
            # double-buffers across chunks with 4 banks each.
            with tc.tile_pool(name="hst", bufs=1) as hstp, \
                 tc.tile_pool(name="wa", bufs=2 * KT + 4) as wap, \
                 tc.tile_pool(name="sqs", bufs=2) as sqsp, \
                 tc.tile_pool(name="psA", bufs=2, space="PSUM") as psp:
                hst = [hstp.tile([128, TPC], BF16, name=f"hst{k}", tag=f"hst{k}")
                       for k in range(KT)]
                # two half-column DMAs per tile, half0 (m 0-3) streamed first so
                # the first chunk's g0 matmuls start as early as possible
                for half in range(2):
                    for k in range(KT):
                        eng = nc.scalar if k % 2 == 0 else nc.gpsimd
                        eng.dma_start(out=hst[k][:, half * 512:(half + 1) * 512],
                                      in_=hsT_d[k * 128:(k + 1) * 128, half * 512:(half + 1) * 512])
                for c, (col0, w) in enumerate(A_CHUNKS):
                    ps0 = psp.tile([128, 4, 512], F32, tag="ps")
                    ps1 = psp.tile([128, 4, 512], F32, tag="ps")
                    wts = []
                    for k in range(KT):
                        wa_t = wap.tile([128, 512], BF16, tag="wa_t")
                        nc.sync.dma_start(out=wa_t[:, :w], in_=waT_d[k * 128:(k + 1) * 128, col0:col0 + w])
                        wts.append(wa_t)
                        for mi in range(4):
                            nc.tensor.matmul(ps0[:, mi, :w], hst[k][:, mi * 128:(mi + 1) * 128],
                                             wa_t[:, :w], start=(k == 0), stop=(k == KT - 1))
                    for k in range(KT):
                        for mi in range(4):
                            m = 4 + mi
                            nc.tensor.matmul(ps1[:, mi, :w], hst[k][:, m * 128:(m + 1) * 128],
                                             wts[k][:, :w], start=(k == 0), stop=(k == KT - 1))
                    # evictions split across the (otherwise idle) vector engine
                    # and scalar, per mi, so no single engine queue backlogs
                    for g, ps in ((0, ps0), (1, ps1)):
                        for mi in range(4):
                            m = g * 4 + mi
                            if c < 4:
                                asl = a_sb[:, m, col0:col0 + w]
                                scr = sqsp.tile([128, 512], F32, tag="scr")
                                if mi % 2 == 0:
                                    nc.vector.tensor_copy(asl, ps[:, mi, :w])
                                else:
                                    nc.scalar.activation(asl, ps[:, mi, :w], AF.Copy)
                                nc.scalar.activation(scr[:, :w], ps[:, mi, :w], AF.Square,
                                                     accum_out=stats[:, m, c:c + 1])
                            else:
                                if mi % 2 == 0:
                                    nc.vector.tensor_copy(kpe_sb[:, m, :], ps[:, mi, :w])
                                else:
                                    nc.scalar.activation(kpe_sb[:, m, :], ps[:, mi, :w], AF.Copy)

            # per-token rsqrt scales (all m at once)
            with tc.tile_pool(name="scl", bufs=1) as sclp:
                for (dst, c0, c1, dim) in ((s_q, 0, 3, RQ), (s_kv, 3, 4, R_KV)):
                    t = sclp.tile([128, MT], F32)
                    nc.vector.reduce_sum(out=t, in_=stats[:, :, c0:c1], axis=AX.X)
                    nc.vector.tensor_scalar(out=t, in0=t, scalar1=1.0 / dim, scalar2=EPS,
                                            op0=ALU.mult, op1=ALU.add)
                    nc.vector.reciprocal(t, t)
                    nc.scalar.activation(dst[:, :], t, AF.Sqrt)

            # ---------------- phase T: transposes,  phase B: B GEMMs ----------------
            with tc.tile_pool(name="at", bufs=1) as atp:
                at_q = atp.tile([128, QKT, TPC], BF16)          # 24KB/p
                at_kv = atp.tile([128, KVKT, TPC], BF16)        # 8KB/p
                ident = atp.tile([128, 128], BF16)
                make_identity(nc, ident)

                def tok_view(h, g, d0, w):
                    # DRAM view [p, mi, w] matching ev tiles' [partition, mi, cols]
                    return out_d[h, g * 512:(g + 1) * 512, d0:d0 + w].rearrange(
                        "(mi p) w -> p mi w", p=128)

                # transposes: a_sb [tok, f] -> at [f, tok]; psum->sbuf copies
                # split across vector and scalar engines, 4 f-tiles per copy
                with tc.tile_pool(name="psT", bufs=2, space="PSUM") as psT:
                    # kv f-tiles (fb 3) first: the kv B GEMM runs before q
                    for fb in (3, 0, 1, 2):
                        pt = psT.tile([128, MT, 512], BF16, tag="pt")
                        for fi in range(4):
                            f = fb * 4 + fi
                            for m in range(MT):
                                nc.tensor.transpose(pt[:, m, fi * 128:(fi + 1) * 128],
                                                    a_sb[:, m, f * 128:(f + 1) * 128], ident)
                        for m in range(MT):
                            src = pt[:, m, :].rearrange("p (a b) -> p a b", b=128)
                            if fb < 3:
                                dst = at_q[:, fb * 4:(fb + 1) * 4, m * 128:(m + 1) * 128]
                            else:
                                dst = at_kv[:, 0:4, m * 128:(m + 1) * 128]
                            if m % 2 == 0:
                                nc.vector.tensor_copy(dst, src)
                            else:
                                nc.scalar.activation(dst, src, AF.Copy)

                out_rings = (nc.gpsimd, nc.scalar)
                ring_i = [0]

                def out_dma(dst, src):
                    out_rings[ring_i[0] % 2].dma_start(out=dst, in_=src)
                    ring_i[0] += 1

                kv_rings = (nc.gpsimd, nc.scalar, nc.sync)
                kv_i = [0]

                def kv_dma(dst, src):
                    kv_rings[kv_i[0] % 3].dma_start(out=dst, in_=src)
                    kv_i[0] += 1

                with tc.tile_pool(name="wb", bufs=2 * QKT + 2) as wbp, \
                     tc.tile_pool(name="ev", bufs=10) as evp, \
                     tc.tile_pool(name="psB", bufs=2, space="PSUM") as psp:
                    for c in range(KVCH):                   # kv chunks (512 cols = 2 heads)
                        ps0 = psp.tile([128, 4, 512], F32, tag="ps")
                        ps1 = psp.tile([128, 4, 512], F32, tag="ps")
                        wts = []
                        for k in range(KVKT):
                            wb_t = wbp.tile([128, 512], BF16, tag="wb_t")
                            nc.sync.dma_start(out=wb_t, in_=kvbT_d[k * 128:(k + 1) * 128, c * 512:(c + 1) * 512])
                            wts.append(wb_t)
                            for mi in range(4):
                                nc.tensor.matmul(ps0[:, mi, :], at_kv[:, k, mi * 128:(mi + 1) * 128],
                                                 wb_t, start=(k == 0), stop=(k == KVKT - 1))
                        for k in range(KVKT):
                            for mi in range(4):
                                m = 4 + mi
                                nc.tensor.matmul(ps1[:, mi, :], at_kv[:, k, m * 128:(m + 1) * 128],
                                                 wts[k], start=(k == 0), stop=(k == KVKT - 1))
                        evs = []
                        for g, ps in ((0, ps0), (1, ps1)):
                            ev = evp.tile([128, 4, 512], F32, tag="ev")
                            evs.append(ev)
                            for mi in range(4):
                                m = g * 4 + mi
                                if mi % 2 == 0:
                                    nc.vector.tensor_scalar_mul(ev[:, mi, :], ps[:, mi, :],
                                                                s_kv[:, m:m + 1])
                                else:
                                    nc.scalar.activation(ev[:, mi, :], ps[:, mi, :], AF.Copy,
                                                         scale=s_kv[:, m:m + 1])
                        # kv outputs are packet-rate bound (512B pieces):
                        # spread them across all three DMA rings; sync's kv
                        # weight stream is tiny so it has packet slack
                        for g, ev in enumerate(evs):
                            for hh in range(2):
                                h = 2 * c + hh
                                kv_dma(tok_view(H + h, g, 0, D_NOPE),
                                       ev[:, :, hh * 256:hh * 256 + 128])
                                kv_dma(tok_view(2 * H + h, g, 0, D_V),
                                       ev[:, :, hh * 256 + 128:hh * 256 + 256])
                    # k_pe broadcast to all key heads (not normed, not scaled):
                    # one DMA per head covering all tokens
                    for h in range(H):
                        nc.gpsimd.dma_start(
                            out=out_d[H + h, :, D_NOPE:D_Q].rearrange("(mi p) w -> p mi w", p=128),
                            in_=kpe_sb[:, :, :])
                    for c in range(QCH):                    # q chunks (512 cols)
                        ps0 = psp.tile([128, 4, 512], F32, tag="ps")
                        ps1 = psp.tile([128, 4, 512], F32, tag="ps")
                        wts = []
                        for k in range(QKT):
                            wb_t = wbp.tile([128, 512], BF16, tag="wb_t")
                            nc.sync.dma_start(out=wb_t, in_=qbT_d[k * 128:(k + 1) * 128, c * 512:(c + 1) * 512])
                            wts.append(wb_t)
                            for mi in range(4):
                                nc.tensor.matmul(ps0[:, mi, :], at_q[:, k, mi * 128:(mi + 1) * 128],
                                                 wb_t, start=(k == 0), stop=(k == QKT - 1))
                        for k in range(QKT):
                            for mi in range(4):
                                m = 4 + mi
                                nc.tensor.matmul(ps1[:, mi, :], at_q[:, k, m * 128:(m + 1) * 128],
                                                 wts[k], start=(k == 0), stop=(k == QKT - 1))
                        evs = []
                        for g, ps in ((0, ps0), (1, ps1)):
                            ev = evp.tile([128, 4, 512], F32, tag="ev")
                            evs.append(ev)
                            for mi in range(4):
                                m = g * 4 + mi
                                if mi % 2 == 0:
                                    nc.vector.tensor_scalar_mul(ev[:, mi, :], ps[:, mi, :],
                                                                s_q[:, m:m + 1])
                                else:
                                    nc.scalar.activation(ev[:, mi, :], ps[:, mi, :], AF.Copy,
                                                         scale=s_q[:, m:m + 1])
                        for g, ev in enumerate(evs):
                            col = c * 512
                            end = col + 512
                            while col < end:                # one DMA per head-piece
                                h = col // D_Q
                                seg_end = min(end, (h + 1) * D_Q)
                                out_dma(tok_view(h, g, col - h * D_Q, seg_end - col),
                                        ev[:, :, col - c * 512:seg_end - c * 512])
                                col = seg_end

    nc.compile()
    _CACHE["nc"] = nc
    return nc


def _prep_inputs(hidden_states, q_a_w, kv_a_w, q_b_w, kv_b_w, q_a_ln_w, kv_a_ln_w):
    import ml_dtypes
    f32 = np.float32
    bf16 = ml_dtypes.bfloat16
    hs = np.asarray(hidden_states, dtype=f32).reshape(NTOK, DMODEL)
    hsT = np.ascontiguousarray(hs.T).astype(bf16)          # [4096, 8192]
    perm = _perm64()

    q_a_w = np.asarray(q_a_w, dtype=f32)
    kv_a_w = np.asarray(kv_a_w, dtype=f32)
    kv_a_pe = kv_a_w[R_KV:][perm]                          # de-interleave k_pe rows
    wa = np.concatenate([q_a_w, kv_a_w[:R_KV], kv_a_pe], axis=0)   # [2112, 4096]
    waT = np.ascontiguousarray(wa.T).astype(bf16)          # [4096, 2112]

    qb = np.asarray(q_b_w, dtype=f32) * np.asarray(q_a_ln_w, dtype=f32)[None, :]
    qb = qb.reshape(H, D_Q, RQ).copy()
    qb[:, D_NOPE:, :] = qb[:, D_NOPE + perm, :]            # de-interleave q_pe rows
    qbT = np.ascontiguousarray(qb.reshape(QOUT, RQ).T).astype(bf16)  # [1536, 6144]

    kvb = np.asarray(kv_b_w, dtype=f32) * np.asarray(kv_a_ln_w, dtype=f32)[None, :]
    kvbT = np.ascontiguousarray(kvb.T).astype(bf16)        # [512, 8192]

    in_maps = []
    for c in range(NCORES):
        in_maps.append({
            "hsT": np.ascontiguousarray(hsT[:, c * TPC:(c + 1) * TPC]),
            "waT": waT,
            "qbT": qbT,
            "kvbT": kvbT,
        })
    return in_maps


def kernel(hidden_states, q_a_w, q_b_w, kv_a_w, kv_b_w, q_a_ln_w, kv_a_ln_w,
           _trace=False):
    _ensure_env()
    from concourse.bass_utils import run_bass_kernel_spmd

    nc = _build()
    in_maps = _prep_inputs(hidden_states, q_a_w, kv_a_w, q_b_w, kv_b_w,
                           q_a_ln_w, kv_a_ln_w)
    res = run_bass_kernel_spmd(nc, in_maps, list(range(NCORES)), trace=_trace)

    out = np.empty((B, 3 * H, S, D_Q), dtype=np.float32)
    for c in range(NCORES):
        out[c // (S // TPC), :, (c % (S // TPC)) * TPC:((c % (S // TPC)) + 1) * TPC, :] = \
            res.results[c]["out"]
    out[:, 2 * H:, :, D_V:] = 0.0      # v padding is exact zeros
    if _trace:
        kernel.last_exec_time_ns = res.exec_time_ns
        kernel.last_results = res
    return out
